# revision 35
# baseline (speedup 1.0000x reference)
"""Trainium2 Bass kernel for nn_DistanceLayer (gaussian-prior distance attention).

Math: out[b,i] = sum_j softmax_j(q_i.k_j * MD^-0.5 * prior(j-i))[j] * (j-i)

Key observation: the gaussian prior (std=1) underflows so fast in f32 that
for |j-i| outside a small band the f32 score is exactly 0, so exp(score)
is exactly 1.0.  The softmax row then consists of a small band of
"interesting" values plus a uniform far field whose sums are known in
closed form.  We therefore compute only a narrow window of scores around
the diagonal on the PE and fold the far field in with exact host-side
constants:

    T0_i = (N - win) + sum_window exp(s)            (denominator)
    T1_i = C1_i + sum_window exp(s)*c + ws_i * sum_window exp(s)
    out_i = T1_i / T0_i - i

where C1_i = sum_all_j j - sum_window_i j (exact integers < 2^24, exact in
f32) and ws_i is the window start of row i's 64-row half-tile.  In-window
far entries have score exactly 0 (prior premultiplied in, 0 outside the
band) and contribute exp(0)=1, which the constants account for.

Layout: rows are processed as 64-row halves packed two-per-partition-dim
(windows stay narrow: win = 64 + band + pad), and two 128-row tiles are
batched per postprocessing pass ([P, 2*win] multiply/exp, 3D reduces for
the per-tile sums) to amortize fixed per-op engine costs.

Sharding: pure data-parallel over batch B=8 across the 8 cores; each core
holds the full (small) QK weights and computes its own [N] output row.
"""

import sys

sys.path.insert(0, "/opt/trn_rl_repo")

import ml_dtypes
import numpy as np

import concourse.bacc as bacc
import concourse.tile as tile
from concourse import mybir
from concourse.bass_utils import run_bass_kernel_spmd

B, N, D, MD = 8, 2048, 256, 128
NCORES = 8
P = 128
HR = P // 2  # 64-row half-tiles
NT = N // P  # 16 row tiles
NPAIR = NT // 2  # 8 postprocessing pairs
DCH = D // P  # 2 contraction chunks for the projections
PROJ_CHUNK = 512
NPC = N // PROJ_CHUNK  # 4 projection column chunks
PI = 3.1415926  # matches reference
F32 = mybir.dt.float32
BF16 = mybir.dt.bfloat16

_cache = {}
# exposed for test harness profiling: (nc, in_maps)
last_run = None


def _plan_band(prior_mean, prior_std):
    """f32 prior over every offset, exactly as the reference computes it,
    and the band of offsets whose scores can round exp() away from 1.0."""
    d = np.arange(-(N - 1), N, dtype=np.float32)
    ps = np.float32(prior_std)
    pm = np.float32(prior_mean)
    prior = (
        np.float32(1.0)
        / ps
        / np.sqrt(np.float32(2.0) * np.float32(PI))
        * np.exp(np.float32(-0.5) * (d - pm) ** 2 / ps**2)
    ).astype(np.float32)
    # |score| <= |prior| * |q.k*scale| ; bound the latter by 1024 (actual
    # max is ~7 for these glorot inputs).  exp(x) rounds to 1.0f for
    # |x| < 2^-26; use 2^-27 for margin.
    sig = np.abs(prior) * 1024.0 >= 2.0**-27
    if not sig.any():
        dlo, dhi = 0, 0
    else:
        dlo = int(d[sig].min())
        dhi = int(d[sig].max())
    return prior, dlo, dhi


def _window_geometry(dlo, dhi):
    """Per-64-row-half window starts ws2[32] plus deduplicated per-pair
    prior patterns.  Pattern key for pair g (tiles 2g, 2g+1) is the tuple
    of its four half-window offsets relative to the pair's base row."""
    span = dhi - dlo
    win = HR + span + 1
    win = max(80, ((win + 15) // 16) * 16)
    assert win <= 512, f"prior band too wide for banded kernel: {dlo}..{dhi}"
    extra = win - (HR + span)
    ws2 = []
    for h in range(2 * NT):
        ws = min(max(h * HR + dlo - extra // 2, 0), N - win)
        lo_need = max(0, h * HR + dlo)
        hi_need = min(N - 1, h * HR + HR - 1 + dhi)
        assert ws <= lo_need and hi_need < ws + win, (h, ws, lo_need, hi_need)
        ws2.append(ws)
    pair_keys = []
    for g in range(NPAIR):
        base = 2 * P * g
        pair_keys.append(tuple(ws2[4 * g + i] - base for i in range(4)))
    key_vals = sorted(set(pair_keys))
    key_idx = [key_vals.index(k) for k in pair_keys]
    return win, ws2, key_vals, key_idx


def _build(win, ws2, key_idx, n_pat):
    nc = bacc.Bacc()

    # f32 consts: bq | bk | c1 | wsm | ii | j0pair ; bf16: pair prior patterns
    CW = 2 + 3 * NT + 2 * win
    O_BQ, O_BK = 0, 1
    O_C1 = 2
    O_WS = O_C1 + NT
    O_II = O_WS + NT
    O_J0 = O_II + NT
    CW16 = n_pat * 2 * win

    w2_d = nc.dram_tensor("w2", [P, 2 * DCH * MD], BF16, kind="ExternalInput")
    xt_d = nc.dram_tensor("xt", [NPC, P, DCH * PROJ_CHUNK], BF16, kind="ExternalInput")
    cs_d = nc.dram_tensor("cst", [P, CW], F32, kind="ExternalInput")
    c16_d = nc.dram_tensor("cst16", [P, CW16], BF16, kind="ExternalInput")
    y_d = nc.dram_tensor("y", [P, NT], F32, kind="ExternalOutput")

    with tile.TileContext(nc) as tc:
        with (
            tc.tile_pool(name="const", bufs=1) as const,
            tc.tile_pool(name="psum_proj", bufs=3, space="PSUM") as psum_proj,
            tc.tile_pool(name="psum_band", bufs=4, space="PSUM") as psum_band,
            tc.tile_pool(name="band_sp", bufs=3) as sp_pool,
            tc.tile_pool(name="band_e", bufs=3) as e_pool,
            tc.tile_pool(name="band_ej", bufs=3) as ej_pool,
            tc.tile_pool(name="comb", bufs=1) as comb,
        ):
            # ---- engine warmups (run while DMAs are in flight) ----
            # PE: junk matmuls keep the PE busy until the input DMAs land,
            # flipping the HAM clock gate to 8/8 before the real matmuls.
            # ACT: one tiny Exp pulls the 1.3us ACT_TABLE_LOAD off the
            # critical path.
            wtile = const.tile([P, PROJ_CHUNK], BF16, tag="warm_w")
            nc.vector.memset(wtile, 0.0)
            for _ in range(7):
                wps = psum_proj.tile([P, PROJ_CHUNK], F32, tag="proj")
                nc.tensor.matmul(
                    wps,
                    lhsT=wtile[:, :P],
                    rhs=wtile[:, :PROJ_CHUNK],
                    start=True,
                    stop=True,
                )
            wact_in = const.tile([P, 1], F32, tag="warm_a")
            nc.vector.memset(wact_in, 0.0)
            wact_out = const.tile([P, 1], F32, tag="warm_ao")
            nc.scalar.activation(
                out=wact_out, in_=wact_in, func=mybir.ActivationFunctionType.Exp
            )

            # ---- input DMAs; first ones go on the scalar queue so their
            # descriptor generation runs parallel to sync's preamble ----
            w2_s = const.tile([P, 2 * DCH * MD], BF16, tag="w2")
            nc.scalar.dma_start(out=w2_s, in_=w2_d[:, :])
            xts = []
            for i in range(NPC):
                t = const.tile([P, DCH * PROJ_CHUNK], BF16, tag=f"xt{i}")
                xts.append(t)
            nc.scalar.dma_start(out=xts[0], in_=xt_d[0])
            nc.scalar.dma_start(out=xts[1], in_=xt_d[1])
            cs_s = const.tile([P, CW], F32, tag="cst")
            nc.sync.dma_start(out=cs_s, in_=cs_d[:, :])
            c16_s = const.tile([P, CW16], BF16, tag="cst16")
            nc.sync.dma_start(out=c16_s, in_=c16_d[:, :])
            nc.sync.dma_start(out=xts[2], in_=xt_d[2])
            nc.gpsimd.dma_start(out=xts[3], in_=xt_d[3])

            qT = const.tile([P, N], BF16, tag="qT")
            kT = const.tile([P, N], BF16, tag="kT")
            sum_e = const.tile([P, NT], F32, tag="sum_e")
            sum_ec = const.tile([P, NT], F32, tag="sum_ec")

            # ---- band pair: tiles 2g, 2g+1 share one [P, 2*win] pass ----
            def emit_pair(g):
                ps_s = psum_band.tile([P, 2 * win], F32, tag="band")
                for tb in range(2):  # tile within pair
                    t = 2 * g + tb
                    for hb in range(2):  # 64-row half on partitions
                        ws = ws2[2 * t + hb]
                        nc.tensor.matmul(
                            ps_s[hb * HR : (hb + 1) * HR, tb * win : (tb + 1) * win],
                            lhsT=qT[:, t * P + hb * HR : t * P + (hb + 1) * HR],
                            rhs=kT[:, ws : ws + win],
                            start=True,
                            stop=True,
                        )
                oi = key_idx[g]
                sp_t = sp_pool.tile([P, 2 * win], F32, tag="sp")
                nc.vector.tensor_mul(
                    sp_t, ps_s, c16_s[:, oi * 2 * win : (oi + 1) * 2 * win]
                )
                e_t = e_pool.tile([P, 2 * win], F32, tag="e")
                nc.scalar.activation(
                    out=e_t, in_=sp_t, func=mybir.ActivationFunctionType.Exp
                )
                ej_t = ej_pool.tile([P, 2 * win], F32, tag="ej")
                mul_eng = nc.vector if g == NPAIR - 1 else nc.gpsimd
                mul_eng.tensor_mul(ej_t, e_t, cs_s[:, O_J0 : O_J0 + 2 * win])
                nc.vector.tensor_reduce(
                    out=sum_e[:, 2 * g : 2 * g + 2],
                    in_=e_t[:].rearrange("p (t w) -> p t w", w=win),
                    axis=mybir.AxisListType.X,
                    op=mybir.AluOpType.add,
                )
                nc.vector.tensor_reduce(
                    out=sum_ec[:, 2 * g : 2 * g + 2],
                    in_=ej_t[:].rearrange("p (t w) -> p t w", w=win),
                    axis=mybir.AxisListType.X,
                    op=mybir.AluOpType.add,
                )

            # pair g needs both projections evicted through this chunk:
            def pair_chunk(g):
                hi = max(min(ws2[h] + win, N) for h in range(4 * g, 4 * g + 4))
                return max((2 * g + 1) // (PROJ_CHUNK // P), (hi - 1) // PROJ_CHUNK)

            pairs_after = {n4: [] for n4 in range(NPC)}
            for g in range(NPAIR):
                pairs_after[pair_chunk(g)].append(g)

            # ---- projections; chunk0 evictions split across ACT+DVE for
            # the fastest band unlock, later chunks all on ACT (the band
            # postprocessing now loads DVE+GpSimd more than ACT) ----
            def emit_chunk(n4, split_evict=False):
                for pj in range(2):  # 0=q, 1=k
                    b_s = cs_s[:, O_BQ + pj : O_BQ + pj + 1]
                    dstT = (qT, kT)[pj]
                    ps_t = psum_proj.tile([P, PROJ_CHUNK], F32, tag="proj")
                    for c in range(DCH):
                        nc.tensor.matmul(
                            ps_t,
                            lhsT=w2_s[:, (2 * pj + c) * MD : (2 * pj + c + 1) * MD],
                            rhs=xts[n4][:, c * PROJ_CHUNK : (c + 1) * PROJ_CHUNK],
                            start=(c == 0),
                            stop=(c == DCH - 1),
                        )
                    lo = n4 * PROJ_CHUNK
                    if split_evict:
                        half = PROJ_CHUNK // 2
                        nc.vector.tensor_scalar_add(
                            dstT[:, lo : lo + half], ps_t[:, :half], b_s
                        )
                        nc.scalar.activation(
                            out=dstT[:, lo + half : lo + PROJ_CHUNK],
                            in_=ps_t[:, half:],
                            func=mybir.ActivationFunctionType.Identity,
                            bias=b_s,
                            scale=1.0,
                        )
                    else:
                        nc.scalar.activation(
                            out=dstT[:, lo : lo + PROJ_CHUNK],
                            in_=ps_t,
                            func=mybir.ActivationFunctionType.Identity,
                            bias=b_s,
                            scale=1.0,
                        )

            # shift-by-one: pair MMs are emitted after the NEXT chunk's
            # matmuls so their evictions are already done (engine queues
            # are FIFO; a waiting matmul would stall the whole PE queue).
            emit_chunk(0, split_evict=True)
            emit_chunk(1)
            band_plan = []
            for n4 in range(2, NPC + 2):
                for g in pairs_after[n4 - 2]:
                    band_plan.append(("pair", g))
                if n4 < NPC:
                    band_plan.append(("chunk", n4))


            # ---- combine: out = (c1 + sum_ec + ws*sum_e)/(N-win+sum_e) - i ----
            c1_s = cs_s[:, O_C1 : O_C1 + NT]
            ws_s = cs_s[:, O_WS : O_WS + NT]
            ii_s = cs_s[:, O_II : O_II + NT]
            outv2 = comb.tile([P, NT], F32, tag="outv2")

            def emit_combine(sl):
                w = sl.stop - sl.start
                t0 = comb.tile([P, w], F32, tag="t0")
                nc.vector.tensor_scalar_add(t0, sum_e[:, sl], float(N - win))
                rec = comb.tile([P, w], F32, tag="rec")
                nc.vector.reciprocal(rec, t0)
                tmp = comb.tile([P, w], F32, tag="tmp")
                nc.vector.tensor_mul(tmp, ws_s[:, sl], sum_e[:, sl])
                num = comb.tile([P, w], F32, tag="num")
                nc.vector.tensor_add(num, c1_s[:, sl], sum_ec[:, sl])
                num2 = comb.tile([P, w], F32, tag="num2")
                nc.vector.tensor_add(num2, num, tmp)
                outv = comb.tile([P, w], F32, tag="outv")
                nc.vector.tensor_mul(outv, num2, rec)
                nc.vector.tensor_sub(outv2[:, sl], outv, ii_s[:, sl])

            # first-half combine hides under the last pairs
            for kind, v in band_plan:
                if kind == "pair":
                    emit_pair(v)
                    if v == NPAIR - 2:
                        emit_combine(slice(0, 8))
                else:
                    emit_chunk(v)
            emit_combine(slice(8, NT))
            nc.sync.dma_start(out=y_d[:, :], in_=outv2)

    nc.finalize()
    return nc


def kernel(x, Wq, bq, Wk, bk, prior_mean, prior_std):
    global last_run
    x = np.asarray(x, dtype=np.float32)
    Wq = np.asarray(Wq, dtype=np.float32)
    Wk = np.asarray(Wk, dtype=np.float32)
    bq = np.asarray(bq, dtype=np.float32)
    bk = np.asarray(bk, dtype=np.float32)

    prior, dlo, dhi = _plan_band(
        float(np.asarray(prior_mean)[0]), float(np.asarray(prior_std)[0])
    )
    win, ws2, key_vals, key_idx = _window_geometry(dlo, dhi)
    n_pat = len(key_vals)

    key = (win, tuple(ws2), tuple(key_idx))
    if key not in _cache:
        _cache[key] = _build(win, ws2, key_idx, n_pat)
    nc = _cache[key]

    bf = ml_dtypes.bfloat16
    scale = np.float32(MD**-0.5)

    # prior*scale pair patterns: [P, 2*win] per distinct 4-offset key.
    # value[p, tb*win + c] = prior[c + rel_ws[tb, hb] - 128*tb - p] * scale
    # where hb selects by partition half (p >= 64).
    p_idx = np.arange(P)[:, None]
    c_idx = np.arange(win)[None, :]
    pmat = np.zeros((P, n_pat * 2 * win), np.float32)
    for ki, rel in enumerate(key_vals):
        for tb in range(2):
            relcol = np.where(np.arange(P) < HR, rel[2 * tb], rel[2 * tb + 1])[:, None]
            dm = c_idx + relcol - 128 * tb - p_idx
            pmat[:, ki * 2 * win + tb * win : ki * 2 * win + (tb + 1) * win] = np.where(
                (dm >= dlo) & (dm <= dhi), prior[dm + N - 1] * scale, np.float32(0.0)
            ).astype(np.float32)

    sumj_all = float(N * (N - 1) // 2)
    c1 = np.zeros((P, NT), np.float32)
    wsm = np.zeros((P, NT), np.float32)
    ii = np.zeros((P, NT), np.float32)
    half_sel = np.arange(P) >= HR
    for t in range(NT):
        wsa, wsb = ws2[2 * t], ws2[2 * t + 1]
        wsv = np.where(half_sel, float(wsb), float(wsa))
        c1[:, t] = sumj_all - (win * wsv + win * (win - 1) // 2)
        wsm[:, t] = wsv
        ii[:, t] = t * P + np.arange(P)

    # consts: f32 = bq | bk | c1 | wsm | ii | j0pair ; bf16 = pair patterns
    j0pair = np.broadcast_to(
        np.tile(np.arange(win, dtype=np.float32), 2), (P, 2 * win)
    )
    cst = np.ascontiguousarray(
        np.concatenate(
            [bq.reshape(P, 1), bk.reshape(P, 1), c1, wsm, ii, j0pair], axis=1
        ).astype(np.float32)
    )
    cst16 = np.ascontiguousarray(pmat.astype(bf))

    # weights: wq chunks then wk chunks, [P, 4*MD]
    wq_h = Wq.reshape(DCH, P, MD).transpose(1, 0, 2).reshape(P, DCH * MD)
    wk_h = Wk.reshape(DCH, P, MD).transpose(1, 0, 2).reshape(P, DCH * MD)
    w2_h = np.ascontiguousarray(np.concatenate([wq_h, wk_h], axis=1)).astype(bf)

    in_maps = []
    for core in range(NCORES):
        xb = x[core]  # [N, D]
        # xt[n4, p, c*512 + j] = x[n4*512 + j, c*128 + p]
        xt_h = np.ascontiguousarray(
            xb.T.reshape(DCH, P, NPC, PROJ_CHUNK)
            .transpose(2, 1, 0, 3)
            .reshape(NPC, P, DCH * PROJ_CHUNK)
        ).astype(bf)
        in_maps.append({"xt": xt_h, "w2": w2_h, "cst": cst, "cst16": cst16})

    res = run_bass_kernel_spmd(nc, in_maps, list(range(NCORES)))
    last_run = (nc, in_maps)
    # y[p, t] = out[128t + p]  ->  out = y.T.flatten()
    out = np.stack(
        [res.results[c]["y"].T.reshape(-1) for c in range(NCORES)], axis=0
    )
    return out.astype(np.float32)


# revision 36
# speedup vs baseline: 1.0343x; 1.0343x over previous
"""Trainium2 Bass kernel for nn_DistanceLayer (gaussian-prior distance attention).

Math: out[b,i] = sum_j softmax_j(q_i.k_j * MD^-0.5 * prior(j-i))[j] * (j-i)

Key observation: the gaussian prior (std=1) underflows so fast in f32 that
for |j-i| outside a small band the f32 score is exactly 0, so exp(score)
is exactly 1.0.  The softmax row then consists of a small band of
"interesting" values plus a uniform far field whose sums are known in
closed form.  We therefore compute only a narrow window of scores around
the diagonal on the PE and fold the far field in with exact host-side
constants:

    T0_i = (N - win) + sum_window exp(s)            (denominator)
    T1_i = C1_i + sum_window exp(s)*c + ws_i * sum_window exp(s)
    out_i = T1_i / T0_i - i

where C1_i = sum_all_j j - sum_window_i j (exact integers < 2^24, exact in
f32) and ws_i is the window start of row i's 64-row half-tile.  In-window
far entries have score exactly 0 (prior premultiplied in, 0 outside the
band) and contribute exp(0)=1, which the constants account for.

Layout: rows are processed as 64-row halves packed two-per-partition-dim
(windows stay narrow: win = 64 + band + pad), and two 128-row tiles are
batched per postprocessing pass ([P, 2*win] multiply/exp, 3D reduces for
the per-tile sums) to amortize fixed per-op engine costs.

Sharding: pure data-parallel over batch B=8 across the 8 cores; each core
holds the full (small) QK weights and computes its own [N] output row.
"""

import sys

sys.path.insert(0, "/opt/trn_rl_repo")

import ml_dtypes
import numpy as np

import concourse.bacc as bacc
import concourse.tile as tile
from concourse import mybir
from concourse.bass_utils import run_bass_kernel_spmd

B, N, D, MD = 8, 2048, 256, 128
NCORES = 8
P = 128
HR = P // 2  # 64-row half-tiles
NT = N // P  # 16 row tiles
NPAIR = NT // 2  # 8 postprocessing pairs
DCH = D // P  # 2 contraction chunks for the projections
PROJ_CHUNK = 512
NPC = N // PROJ_CHUNK  # 4 projection column chunks
PI = 3.1415926  # matches reference
F32 = mybir.dt.float32
BF16 = mybir.dt.bfloat16

_cache = {}
# exposed for test harness profiling: (nc, in_maps)
last_run = None


def _plan_band(prior_mean, prior_std):
    """f32 prior over every offset, exactly as the reference computes it,
    and the band of offsets whose scores can round exp() away from 1.0."""
    d = np.arange(-(N - 1), N, dtype=np.float32)
    ps = np.float32(prior_std)
    pm = np.float32(prior_mean)
    prior = (
        np.float32(1.0)
        / ps
        / np.sqrt(np.float32(2.0) * np.float32(PI))
        * np.exp(np.float32(-0.5) * (d - pm) ** 2 / ps**2)
    ).astype(np.float32)
    # |score| <= |prior| * |q.k*scale| ; bound the latter by 1024 (actual
    # max is ~7 for these glorot inputs).  exp(x) rounds to 1.0f for
    # |x| < 2^-26; use 2^-27 for margin.
    sig = np.abs(prior) * 1024.0 >= 2.0**-27
    if not sig.any():
        dlo, dhi = 0, 0
    else:
        dlo = int(d[sig].min())
        dhi = int(d[sig].max())
    return prior, dlo, dhi


def _window_geometry(dlo, dhi):
    """Per-64-row-half window starts ws2[32] plus deduplicated per-pair
    prior patterns.  Pattern key for pair g (tiles 2g, 2g+1) is the tuple
    of its four half-window offsets relative to the pair's base row."""
    span = dhi - dlo
    win = HR + span + 1
    win = max(80, ((win + 15) // 16) * 16)
    assert win <= 512, f"prior band too wide for banded kernel: {dlo}..{dhi}"
    extra = win - (HR + span)
    ws2 = []
    for h in range(2 * NT):
        ws = min(max(h * HR + dlo - extra // 2, 0), N - win)
        lo_need = max(0, h * HR + dlo)
        hi_need = min(N - 1, h * HR + HR - 1 + dhi)
        assert ws <= lo_need and hi_need < ws + win, (h, ws, lo_need, hi_need)
        ws2.append(ws)
    pair_keys = []
    for g in range(NPAIR):
        base = 2 * P * g
        pair_keys.append(tuple(ws2[4 * g + i] - base for i in range(4)))
    key_vals = sorted(set(pair_keys))
    key_idx = [key_vals.index(k) for k in pair_keys]
    return win, ws2, key_vals, key_idx


def _build(win, ws2, key_idx, n_pat):
    nc = bacc.Bacc()

    # f32 consts: bq | bk | c1 | wsm | ii | j0pair ; bf16: pair prior patterns
    CW = 2 + 3 * NT + 2 * win
    O_BQ, O_BK = 0, 1
    O_C1 = 2
    O_WS = O_C1 + NT
    O_II = O_WS + NT
    O_J0 = O_II + NT
    CW16 = n_pat * 2 * win

    w2_d = nc.dram_tensor("w2", [P, 2 * DCH * MD], BF16, kind="ExternalInput")
    xt_d = nc.dram_tensor("xt", [NPC, P, DCH * PROJ_CHUNK], BF16, kind="ExternalInput")
    cs_d = nc.dram_tensor("cst", [P, CW], F32, kind="ExternalInput")
    c16_d = nc.dram_tensor("cst16", [P, CW16], BF16, kind="ExternalInput")
    y_d = nc.dram_tensor("y", [P, NT], F32, kind="ExternalOutput")

    with tile.TileContext(nc) as tc:
        with (
            tc.tile_pool(name="const", bufs=1) as const,
            tc.tile_pool(name="psum_proj", bufs=3, space="PSUM") as psum_proj,
            tc.tile_pool(name="psum_band", bufs=3, space="PSUM") as psum_band,
            tc.tile_pool(name="band_sp", bufs=2) as sp_pool,
            tc.tile_pool(name="band_e", bufs=2) as e_pool,
            tc.tile_pool(name="band_ej", bufs=2) as ej_pool,
            tc.tile_pool(name="comb", bufs=1) as comb,
        ):
            # ---- engine warmups (run while DMAs are in flight) ----
            # PE: junk matmuls keep the PE busy until the input DMAs land,
            # flipping the HAM clock gate to 8/8 before the real matmuls.
            # ACT: one tiny Exp pulls the 1.3us ACT_TABLE_LOAD off the
            # critical path.
            wtile = const.tile([P, PROJ_CHUNK], BF16, tag="warm_w")
            nc.vector.memset(wtile, 0.0)
            for _ in range(7):
                wps = psum_proj.tile([P, PROJ_CHUNK], F32, tag="proj")
                nc.tensor.matmul(
                    wps,
                    lhsT=wtile[:, :P],
                    rhs=wtile[:, :PROJ_CHUNK],
                    start=True,
                    stop=True,
                )
            wact_in = const.tile([P, 1], F32, tag="warm_a")
            nc.vector.memset(wact_in, 0.0)
            wact_out = const.tile([P, 1], F32, tag="warm_ao")
            nc.scalar.activation(
                out=wact_out, in_=wact_in, func=mybir.ActivationFunctionType.Exp
            )

            # ---- input DMAs; first ones go on the scalar queue so their
            # descriptor generation runs parallel to sync's preamble ----
            w2_s = const.tile([P, 2 * DCH * MD], BF16, tag="w2")
            nc.scalar.dma_start(out=w2_s, in_=w2_d[:, :])
            xts = []
            for i in range(NPC):
                t = const.tile([P, DCH * PROJ_CHUNK], BF16, tag=f"xt{i}")
                xts.append(t)
            nc.scalar.dma_start(out=xts[0], in_=xt_d[0])
            cs_s = const.tile([P, CW], F32, tag="cst")
            nc.scalar.dma_start(out=cs_s, in_=cs_d[:, :])
            c16_s = const.tile([P, CW16], BF16, tag="cst16")
            nc.scalar.dma_start(out=c16_s, in_=c16_d[:, :])
            for i in range(1, NPC):
                nc.sync.dma_start(out=xts[i], in_=xt_d[i])

            qT = const.tile([P, N], BF16, tag="qT")
            kT = const.tile([P, N], BF16, tag="kT")
            sum_e = const.tile([P, NT], F32, tag="sum_e")
            sum_ec = const.tile([P, NT], F32, tag="sum_ec")

            # ---- band pair: tiles 2g, 2g+1 share one [P, 2*win] pass ----
            def emit_pair(g):
                ps_s = psum_band.tile([P, 2 * win], F32, tag="band")
                for tb in range(2):  # tile within pair
                    t = 2 * g + tb
                    for hb in range(2):  # 64-row half on partitions
                        ws = ws2[2 * t + hb]
                        nc.tensor.matmul(
                            ps_s[hb * HR : (hb + 1) * HR, tb * win : (tb + 1) * win],
                            lhsT=qT[:, t * P + hb * HR : t * P + (hb + 1) * HR],
                            rhs=kT[:, ws : ws + win],
                            start=True,
                            stop=True,
                        )
                oi = key_idx[g]
                sp_t = sp_pool.tile([P, 2 * win], F32, tag="sp")
                nc.vector.tensor_mul(
                    sp_t, ps_s, c16_s[:, oi * 2 * win : (oi + 1) * 2 * win]
                )
                e_t = e_pool.tile([P, 2 * win], F32, tag="e")
                nc.scalar.activation(
                    out=e_t, in_=sp_t, func=mybir.ActivationFunctionType.Exp
                )
                ej_t = ej_pool.tile([P, 2 * win], F32, tag="ej")
                mul_eng = nc.vector if g == NPAIR - 1 else nc.gpsimd
                mul_eng.tensor_mul(ej_t, e_t, cs_s[:, O_J0 : O_J0 + 2 * win])
                nc.vector.tensor_reduce(
                    out=sum_e[:, 2 * g : 2 * g + 2],
                    in_=e_t[:].rearrange("p (t w) -> p t w", w=win),
                    axis=mybir.AxisListType.X,
                    op=mybir.AluOpType.add,
                )
                nc.vector.tensor_reduce(
                    out=sum_ec[:, 2 * g : 2 * g + 2],
                    in_=ej_t[:].rearrange("p (t w) -> p t w", w=win),
                    axis=mybir.AxisListType.X,
                    op=mybir.AluOpType.add,
                )

            # pair g needs both projections evicted through this chunk:
            def pair_chunk(g):
                hi = max(min(ws2[h] + win, N) for h in range(4 * g, 4 * g + 4))
                return max((2 * g + 1) // (PROJ_CHUNK // P), (hi - 1) // PROJ_CHUNK)

            pairs_after = {n4: [] for n4 in range(NPC)}
            for g in range(NPAIR):
                pairs_after[pair_chunk(g)].append(g)

            # ---- projections; chunk0 evictions split across ACT+DVE for
            # the fastest band unlock, later chunks all on ACT (the band
            # postprocessing now loads DVE+GpSimd more than ACT) ----
            def emit_chunk(n4, split_evict=False):
                for pj in range(2):  # 0=q, 1=k
                    b_s = cs_s[:, O_BQ + pj : O_BQ + pj + 1]
                    dstT = (qT, kT)[pj]
                    ps_t = psum_proj.tile([P, PROJ_CHUNK], F32, tag="proj")
                    for c in range(DCH):
                        nc.tensor.matmul(
                            ps_t,
                            lhsT=w2_s[:, (2 * pj + c) * MD : (2 * pj + c + 1) * MD],
                            rhs=xts[n4][:, c * PROJ_CHUNK : (c + 1) * PROJ_CHUNK],
                            start=(c == 0),
                            stop=(c == DCH - 1),
                        )
                    lo = n4 * PROJ_CHUNK
                    if split_evict:
                        half = PROJ_CHUNK // 2
                        nc.vector.tensor_scalar_add(
                            dstT[:, lo : lo + half], ps_t[:, :half], b_s
                        )
                        nc.scalar.activation(
                            out=dstT[:, lo + half : lo + PROJ_CHUNK],
                            in_=ps_t[:, half:],
                            func=mybir.ActivationFunctionType.Identity,
                            bias=b_s,
                            scale=1.0,
                        )
                    else:
                        nc.scalar.activation(
                            out=dstT[:, lo : lo + PROJ_CHUNK],
                            in_=ps_t,
                            func=mybir.ActivationFunctionType.Identity,
                            bias=b_s,
                            scale=1.0,
                        )

            # shift-by-one: pair MMs are emitted after the NEXT chunk's
            # matmuls so their evictions are already done (engine queues
            # are FIFO; a waiting matmul would stall the whole PE queue).
            emit_chunk(0, split_evict=True)
            emit_chunk(1)
            band_plan = []
            for n4 in range(2, NPC + 2):
                for g in pairs_after[n4 - 2]:
                    band_plan.append(("pair", g))
                if n4 < NPC:
                    band_plan.append(("chunk", n4))


            # ---- combine: out = (c1 + sum_ec + ws*sum_e)/(N-win+sum_e) - i ----
            c1_s = cs_s[:, O_C1 : O_C1 + NT]
            ws_s = cs_s[:, O_WS : O_WS + NT]
            ii_s = cs_s[:, O_II : O_II + NT]
            outv2 = comb.tile([P, NT], F32, tag="outv2")

            def emit_combine(sl):
                w = sl.stop - sl.start
                t0 = comb.tile([P, w], F32, tag="t0")
                nc.vector.tensor_scalar_add(t0, sum_e[:, sl], float(N - win))
                rec = comb.tile([P, w], F32, tag="rec")
                nc.vector.reciprocal(rec, t0)
                tmp = comb.tile([P, w], F32, tag="tmp")
                nc.vector.tensor_mul(tmp, ws_s[:, sl], sum_e[:, sl])
                num = comb.tile([P, w], F32, tag="num")
                nc.vector.tensor_add(num, c1_s[:, sl], sum_ec[:, sl])
                num2 = comb.tile([P, w], F32, tag="num2")
                nc.vector.tensor_add(num2, num, tmp)
                outv = comb.tile([P, w], F32, tag="outv")
                nc.vector.tensor_mul(outv, num2, rec)
                nc.vector.tensor_sub(outv2[:, sl], outv, ii_s[:, sl])

            # first-half combine hides under the last pairs
            for kind, v in band_plan:
                if kind == "pair":
                    emit_pair(v)
                    if v == NPAIR - 2:
                        emit_combine(slice(0, 8))
                else:
                    emit_chunk(v)
            emit_combine(slice(8, NT))
            nc.sync.dma_start(out=y_d[:, :], in_=outv2)

    nc.finalize()
    return nc


def kernel(x, Wq, bq, Wk, bk, prior_mean, prior_std):
    global last_run
    x = np.asarray(x, dtype=np.float32)
    Wq = np.asarray(Wq, dtype=np.float32)
    Wk = np.asarray(Wk, dtype=np.float32)
    bq = np.asarray(bq, dtype=np.float32)
    bk = np.asarray(bk, dtype=np.float32)

    prior, dlo, dhi = _plan_band(
        float(np.asarray(prior_mean)[0]), float(np.asarray(prior_std)[0])
    )
    win, ws2, key_vals, key_idx = _window_geometry(dlo, dhi)
    n_pat = len(key_vals)

    key = (win, tuple(ws2), tuple(key_idx))
    if key not in _cache:
        _cache[key] = _build(win, ws2, key_idx, n_pat)
    nc = _cache[key]

    bf = ml_dtypes.bfloat16
    scale = np.float32(MD**-0.5)

    # prior*scale pair patterns: [P, 2*win] per distinct 4-offset key.
    # value[p, tb*win + c] = prior[c + rel_ws[tb, hb] - 128*tb - p] * scale
    # where hb selects by partition half (p >= 64).
    p_idx = np.arange(P)[:, None]
    c_idx = np.arange(win)[None, :]
    pmat = np.zeros((P, n_pat * 2 * win), np.float32)
    for ki, rel in enumerate(key_vals):
        for tb in range(2):
            relcol = np.where(np.arange(P) < HR, rel[2 * tb], rel[2 * tb + 1])[:, None]
            dm = c_idx + relcol - 128 * tb - p_idx
            pmat[:, ki * 2 * win + tb * win : ki * 2 * win + (tb + 1) * win] = np.where(
                (dm >= dlo) & (dm <= dhi), prior[dm + N - 1] * scale, np.float32(0.0)
            ).astype(np.float32)

    sumj_all = float(N * (N - 1) // 2)
    c1 = np.zeros((P, NT), np.float32)
    wsm = np.zeros((P, NT), np.float32)
    ii = np.zeros((P, NT), np.float32)
    half_sel = np.arange(P) >= HR
    for t in range(NT):
        wsa, wsb = ws2[2 * t], ws2[2 * t + 1]
        wsv = np.where(half_sel, float(wsb), float(wsa))
        c1[:, t] = sumj_all - (win * wsv + win * (win - 1) // 2)
        wsm[:, t] = wsv
        ii[:, t] = t * P + np.arange(P)

    # consts: f32 = bq | bk | c1 | wsm | ii | j0pair ; bf16 = pair patterns
    j0pair = np.broadcast_to(
        np.tile(np.arange(win, dtype=np.float32), 2), (P, 2 * win)
    )
    cst = np.ascontiguousarray(
        np.concatenate(
            [bq.reshape(P, 1), bk.reshape(P, 1), c1, wsm, ii, j0pair], axis=1
        ).astype(np.float32)
    )
    cst16 = np.ascontiguousarray(pmat.astype(bf))

    # weights: wq chunks then wk chunks, [P, 4*MD]
    wq_h = Wq.reshape(DCH, P, MD).transpose(1, 0, 2).reshape(P, DCH * MD)
    wk_h = Wk.reshape(DCH, P, MD).transpose(1, 0, 2).reshape(P, DCH * MD)
    w2_h = np.ascontiguousarray(np.concatenate([wq_h, wk_h], axis=1)).astype(bf)

    in_maps = []
    for core in range(NCORES):
        xb = x[core]  # [N, D]
        # xt[n4, p, c*512 + j] = x[n4*512 + j, c*128 + p]
        xt_h = np.ascontiguousarray(
            xb.T.reshape(DCH, P, NPC, PROJ_CHUNK)
            .transpose(2, 1, 0, 3)
            .reshape(NPC, P, DCH * PROJ_CHUNK)
        ).astype(bf)
        in_maps.append({"xt": xt_h, "w2": w2_h, "cst": cst, "cst16": cst16})

    res = run_bass_kernel_spmd(nc, in_maps, list(range(NCORES)))
    last_run = (nc, in_maps)
    # y[p, t] = out[128t + p]  ->  out = y.T.flatten()
    out = np.stack(
        [res.results[c]["y"].T.reshape(-1) for c in range(NCORES)], axis=0
    )
    return out.astype(np.float32)


# revision 38
# speedup vs baseline: 1.0423x; 1.0077x over previous
"""Trainium2 Bass kernel for nn_DistanceLayer (gaussian-prior distance attention).

Math: out[b,i] = sum_j softmax_j(q_i.k_j * MD^-0.5 * prior(j-i))[j] * (j-i)

Key observation: the gaussian prior (std=1) underflows so fast in f32 that
for |j-i| outside a small band the f32 score is exactly 0, so exp(score)
is exactly 1.0.  The softmax row then consists of a small band of
"interesting" values plus a uniform far field whose sums are known in
closed form.  We therefore compute only a narrow window of scores around
the diagonal on the PE and fold the far field in with exact host-side
constants:

    T0_i = (N - win) + sum_window exp(s)            (denominator)
    T1_i = C1_i + sum_window exp(s)*c + ws_i * sum_window exp(s)
    out_i = T1_i / T0_i - i

where C1_i = sum_all_j j - sum_window_i j (exact integers < 2^24, exact in
f32) and ws_i is the window start of row i's 64-row half-tile.  In-window
far entries have score exactly 0 (prior premultiplied in, 0 outside the
band) and contribute exp(0)=1, which the constants account for.

Layout: rows are processed as 64-row halves packed two-per-partition-dim
(windows stay narrow: win = 64 + band + pad), and two 128-row tiles are
batched per postprocessing pass ([P, 2*win] multiply/exp, 3D reduces for
the per-tile sums) to amortize fixed per-op engine costs.

Sharding: pure data-parallel over batch B=8 across the 8 cores; each core
holds the full (small) QK weights and computes its own [N] output row.
"""

import sys

sys.path.insert(0, "/opt/trn_rl_repo")

import ml_dtypes
import numpy as np

import concourse.bacc as bacc
import concourse.tile as tile
from concourse import mybir
from concourse.bass_utils import run_bass_kernel_spmd

B, N, D, MD = 8, 2048, 256, 128
NCORES = 8
P = 128
HR = P // 2  # 64-row half-tiles
NT = N // P  # 16 row tiles
NPAIR = NT // 2  # 8 postprocessing pairs
DCH = D // P  # 2 contraction chunks for the projections
PROJ_CHUNK = 512
NPC = N // PROJ_CHUNK  # 4 projection column chunks
PI = 3.1415926  # matches reference
F32 = mybir.dt.float32
BF16 = mybir.dt.bfloat16

_cache = {}
# exposed for test harness profiling: (nc, in_maps)
last_run = None


def _plan_band(prior_mean, prior_std):
    """f32 prior over every offset, exactly as the reference computes it,
    and the band of offsets whose scores can round exp() away from 1.0."""
    d = np.arange(-(N - 1), N, dtype=np.float32)
    ps = np.float32(prior_std)
    pm = np.float32(prior_mean)
    prior = (
        np.float32(1.0)
        / ps
        / np.sqrt(np.float32(2.0) * np.float32(PI))
        * np.exp(np.float32(-0.5) * (d - pm) ** 2 / ps**2)
    ).astype(np.float32)
    # |score| <= |prior| * |q.k*scale| ; bound the latter by 1024 (actual
    # max is ~7 for these glorot inputs).  exp(x) rounds to 1.0f for
    # |x| < 2^-26; use 2^-27 for margin.
    sig = np.abs(prior) * 1024.0 >= 2.0**-27
    if not sig.any():
        dlo, dhi = 0, 0
    else:
        dlo = int(d[sig].min())
        dhi = int(d[sig].max())
    return prior, dlo, dhi


def _window_geometry(dlo, dhi):
    """Per-64-row-half window starts ws2[32] plus deduplicated per-pair
    prior patterns.  Pattern key for pair g (tiles 2g, 2g+1) is the tuple
    of its four half-window offsets relative to the pair's base row."""
    span = dhi - dlo
    win = HR + span + 1
    win = max(80, ((win + 15) // 16) * 16)
    assert win <= 512, f"prior band too wide for banded kernel: {dlo}..{dhi}"
    extra = win - (HR + span)
    ws2 = []
    for h in range(2 * NT):
        ws = min(max(h * HR + dlo - extra // 2, 0), N - win)
        lo_need = max(0, h * HR + dlo)
        hi_need = min(N - 1, h * HR + HR - 1 + dhi)
        assert ws <= lo_need and hi_need < ws + win, (h, ws, lo_need, hi_need)
        ws2.append(ws)
    pair_keys = []
    for g in range(NPAIR):
        base = 2 * P * g
        pair_keys.append(tuple(ws2[4 * g + i] - base for i in range(4)))
    key_vals = sorted(set(pair_keys))
    key_idx = [key_vals.index(k) for k in pair_keys]
    return win, ws2, key_vals, key_idx


def _build(win, ws2, key_idx, n_pat):
    nc = bacc.Bacc()

    # f32 consts: bq | bk | c1 | wsm | ii | j0pair ; bf16: pair prior patterns
    CW = 2 + 3 * NT + 2 * win
    O_BQ, O_BK = 0, 1
    O_C1 = 2
    O_WS = O_C1 + NT
    O_II = O_WS + NT
    O_J0 = O_II + NT
    CW16 = n_pat * 2 * win

    w2_d = nc.dram_tensor("w2", [P, 2 * DCH * MD], BF16, kind="ExternalInput")
    xt_d = nc.dram_tensor("xt", [NPC, P, DCH * PROJ_CHUNK], BF16, kind="ExternalInput")
    cs_d = nc.dram_tensor("cst", [P, CW], F32, kind="ExternalInput")
    c16_d = nc.dram_tensor("cst16", [P, CW16], BF16, kind="ExternalInput")
    y_d = nc.dram_tensor("y", [P, NT], F32, kind="ExternalOutput")

    with tile.TileContext(nc) as tc:
        with (
            tc.tile_pool(name="const", bufs=1) as const,
            tc.tile_pool(name="psum_proj", bufs=3, space="PSUM") as psum_proj,
            tc.tile_pool(name="psum_band", bufs=3, space="PSUM") as psum_band,
            tc.tile_pool(name="band_sp", bufs=2) as sp_pool,
            tc.tile_pool(name="band_e", bufs=2) as e_pool,
            tc.tile_pool(name="band_ej", bufs=2) as ej_pool,
            tc.tile_pool(name="comb", bufs=1) as comb,
        ):
            # ---- engine warmups (run while DMAs are in flight) ----
            # PE: junk matmuls keep the PE busy until the input DMAs land,
            # flipping the HAM clock gate to 8/8 before the real matmuls.
            # ACT: one tiny Exp pulls the 1.3us ACT_TABLE_LOAD off the
            # critical path.
            wtile = const.tile([P, PROJ_CHUNK], BF16, tag="warm_w")
            nc.vector.memset(wtile, 0.0)
            for _ in range(7):
                wps = psum_proj.tile([P, PROJ_CHUNK], F32, tag="proj")
                nc.tensor.matmul(
                    wps,
                    lhsT=wtile[:, :P],
                    rhs=wtile[:, :PROJ_CHUNK],
                    start=True,
                    stop=True,
                )
            wact_in = const.tile([P, 1], F32, tag="warm_a")
            nc.vector.memset(wact_in, 0.0)
            wact_out = const.tile([P, 1], F32, tag="warm_ao")
            nc.scalar.activation(
                out=wact_out, in_=wact_in, func=mybir.ActivationFunctionType.Exp
            )

            # ---- input DMAs; first ones go on the scalar queue so their
            # descriptor generation runs parallel to sync's preamble ----
            w2_s = const.tile([P, 2 * DCH * MD], BF16, tag="w2")
            nc.scalar.dma_start(out=w2_s, in_=w2_d[:, :])
            xts = []
            for i in range(NPC):
                t = const.tile([P, DCH * PROJ_CHUNK], BF16, tag=f"xt{i}")
                xts.append(t)
            nc.scalar.dma_start(out=xts[0], in_=xt_d[0])
            cs_s = const.tile([P, CW], F32, tag="cst")
            nc.scalar.dma_start(out=cs_s, in_=cs_d[:, :])
            c16_s = const.tile([P, CW16], BF16, tag="cst16")
            nc.scalar.dma_start(out=c16_s, in_=c16_d[:, :])
            for i in range(1, NPC):
                nc.sync.dma_start(out=xts[i], in_=xt_d[i])

            qT = const.tile([P, N], BF16, tag="qT")
            kT = const.tile([P, N], BF16, tag="kT")
            sum_e = const.tile([P, NT], F32, tag="sum_e")
            sum_ec = const.tile([P, NT], F32, tag="sum_ec")

            # ---- band pair: tiles 2g, 2g+1 share one [P, 2*win] pass ----
            def emit_pair(g):
                ps_s = psum_band.tile([P, 2 * win], F32, tag="band")
                for tb in range(2):  # tile within pair
                    t = 2 * g + tb
                    for hb in range(2):  # 64-row half on partitions
                        ws = ws2[2 * t + hb]
                        nc.tensor.matmul(
                            ps_s[hb * HR : (hb + 1) * HR, tb * win : (tb + 1) * win],
                            lhsT=qT[:, t * P + hb * HR : t * P + (hb + 1) * HR],
                            rhs=kT[:, ws : ws + win],
                            start=True,
                            stop=True,
                        )
                oi = key_idx[g]
                sp_t = sp_pool.tile([P, 2 * win], F32, tag="sp")
                nc.vector.tensor_mul(
                    sp_t, ps_s, c16_s[:, oi * 2 * win : (oi + 1) * 2 * win]
                )
                e_t = e_pool.tile([P, 2 * win], F32, tag="e")
                nc.scalar.activation(
                    out=e_t, in_=sp_t, func=mybir.ActivationFunctionType.Exp
                )
                ej_t = ej_pool.tile([P, 2 * win], F32, tag="ej")
                mul_eng = nc.vector if g == NPAIR - 1 else nc.gpsimd
                mul_eng.tensor_mul(ej_t, e_t, cs_s[:, O_J0 : O_J0 + 2 * win])
                nc.vector.tensor_reduce(
                    out=sum_e[:, 2 * g : 2 * g + 2],
                    in_=e_t[:].rearrange("p (t w) -> p t w", w=win),
                    axis=mybir.AxisListType.X,
                    op=mybir.AluOpType.add,
                )
                nc.vector.tensor_reduce(
                    out=sum_ec[:, 2 * g : 2 * g + 2],
                    in_=ej_t[:].rearrange("p (t w) -> p t w", w=win),
                    axis=mybir.AxisListType.X,
                    op=mybir.AluOpType.add,
                )

            # pair g needs both projections evicted through this chunk:
            def pair_chunk(g):
                hi = max(min(ws2[h] + win, N) for h in range(4 * g, 4 * g + 4))
                return max((2 * g + 1) // (PROJ_CHUNK // P), (hi - 1) // PROJ_CHUNK)

            pairs_after = {n4: [] for n4 in range(NPC)}
            for g in range(NPAIR):
                pairs_after[pair_chunk(g)].append(g)

            # ---- projections; chunk0 evictions split across ACT+DVE for
            # the fastest band unlock, later chunks all on ACT (the band
            # postprocessing now loads DVE+GpSimd more than ACT) ----
            def emit_chunk(n4, split_evict=False):
                for pj in range(2):  # 0=q, 1=k
                    b_s = cs_s[:, O_BQ + pj : O_BQ + pj + 1]
                    dstT = (qT, kT)[pj]
                    ps_t = psum_proj.tile([P, PROJ_CHUNK], F32, tag="proj")
                    for c in range(DCH):
                        nc.tensor.matmul(
                            ps_t,
                            lhsT=w2_s[:, (2 * pj + c) * MD : (2 * pj + c + 1) * MD],
                            rhs=xts[n4][:, c * PROJ_CHUNK : (c + 1) * PROJ_CHUNK],
                            start=(c == 0),
                            stop=(c == DCH - 1),
                        )
                    lo = n4 * PROJ_CHUNK
                    if split_evict:
                        half = PROJ_CHUNK // 2
                        nc.vector.tensor_scalar_add(
                            dstT[:, lo : lo + half], ps_t[:, :half], b_s
                        )
                        nc.scalar.activation(
                            out=dstT[:, lo + half : lo + PROJ_CHUNK],
                            in_=ps_t[:, half:],
                            func=mybir.ActivationFunctionType.Identity,
                            bias=b_s,
                            scale=1.0,
                        )
                    else:
                        nc.scalar.activation(
                            out=dstT[:, lo : lo + PROJ_CHUNK],
                            in_=ps_t,
                            func=mybir.ActivationFunctionType.Identity,
                            bias=b_s,
                            scale=1.0,
                        )

            # shift-by-one: pair MMs are emitted after the NEXT chunk's
            # matmuls so their evictions are already done (engine queues
            # are FIFO; a waiting matmul would stall the whole PE queue).
            emit_chunk(0, split_evict=True)
            emit_chunk(1)
            band_plan = []
            for n4 in range(2, NPC + 2):
                for g in pairs_after[n4 - 2]:
                    band_plan.append(("pair", g))
                if n4 < NPC:
                    band_plan.append(("chunk", n4))


            # ---- combine: out = (c1 + sum_ec + ws*sum_e)/(N-win+sum_e) - i ----
            c1_s = cs_s[:, O_C1 : O_C1 + NT]
            ws_s = cs_s[:, O_WS : O_WS + NT]
            ii_s = cs_s[:, O_II : O_II + NT]
            outv2 = comb.tile([P, NT], F32, tag="outv2")

            def emit_combine(sl):
                w = sl.stop - sl.start
                t0 = comb.tile([P, w], F32, tag="t0")
                nc.vector.tensor_scalar_add(t0, sum_e[:, sl], float(N - win))
                rec = comb.tile([P, w], F32, tag="rec")
                nc.vector.reciprocal(rec, t0)
                tmp = comb.tile([P, w], F32, tag="tmp")
                nc.vector.tensor_mul(tmp, ws_s[:, sl], sum_e[:, sl])
                num = comb.tile([P, w], F32, tag="num")
                nc.vector.tensor_add(num, c1_s[:, sl], sum_ec[:, sl])
                num2 = comb.tile([P, w], F32, tag="num2")
                nc.vector.tensor_add(num2, num, tmp)
                outv = comb.tile([P, w], F32, tag="outv")
                nc.vector.tensor_mul(outv, num2, rec)
                nc.vector.tensor_sub(outv2[:, sl], outv, ii_s[:, sl])

            # first-half combine hides under the last pairs
            for kind, v in band_plan:
                if kind == "pair":
                    emit_pair(v)
                    if v == NPAIR - 2:
                        emit_combine(slice(0, 8))
                else:
                    emit_chunk(v)
            emit_combine(slice(8, NT))
            nc.sync.dma_start(out=y_d[:, :], in_=outv2)

    nc.finalize()
    return nc


def kernel(x, Wq, bq, Wk, bk, prior_mean, prior_std):
    global last_run
    x = np.asarray(x, dtype=np.float32)
    Wq = np.asarray(Wq, dtype=np.float32)
    Wk = np.asarray(Wk, dtype=np.float32)
    bq = np.asarray(bq, dtype=np.float32)
    bk = np.asarray(bk, dtype=np.float32)

    prior, dlo, dhi = _plan_band(
        float(np.asarray(prior_mean)[0]), float(np.asarray(prior_std)[0])
    )
    win, ws2, key_vals, key_idx = _window_geometry(dlo, dhi)
    n_pat = len(key_vals)

    key = (win, tuple(ws2), tuple(key_idx))
    if key not in _cache:
        _cache[key] = _build(win, ws2, key_idx, n_pat)
    nc = _cache[key]

    bf = ml_dtypes.bfloat16
    scale = np.float32(MD**-0.5)

    # prior*scale pair patterns: [P, 2*win] per distinct 4-offset key.
    # value[p, tb*win + c] = prior[c + rel_ws[tb, hb] - 128*tb - p] * scale
    # where hb selects by partition half (p >= 64).
    p_idx = np.arange(P)[:, None]
    c_idx = np.arange(win)[None, :]
    pmat = np.zeros((P, n_pat * 2 * win), np.float32)
    for ki, rel in enumerate(key_vals):
        for tb in range(2):
            relcol = np.where(np.arange(P) < HR, rel[2 * tb], rel[2 * tb + 1])[:, None]
            dm = c_idx + relcol - 128 * tb - p_idx
            pmat[:, ki * 2 * win + tb * win : ki * 2 * win + (tb + 1) * win] = np.where(
                (dm >= dlo) & (dm <= dhi), prior[dm + N - 1] * scale, np.float32(0.0)
            ).astype(np.float32)

    sumj_all = float(N * (N - 1) // 2)
    c1 = np.zeros((P, NT), np.float32)
    wsm = np.zeros((P, NT), np.float32)
    ii = np.zeros((P, NT), np.float32)
    half_sel = np.arange(P) >= HR
    for t in range(NT):
        wsa, wsb = ws2[2 * t], ws2[2 * t + 1]
        wsv = np.where(half_sel, float(wsb), float(wsa))
        c1[:, t] = sumj_all - (win * wsv + win * (win - 1) // 2)
        wsm[:, t] = wsv
        ii[:, t] = t * P + np.arange(P)

    # consts: f32 = bq | bk | c1 | wsm | ii | j0pair ; bf16 = pair patterns
    j0pair = np.broadcast_to(
        np.tile(np.arange(win, dtype=np.float32), 2), (P, 2 * win)
    )
    cst = np.ascontiguousarray(
        np.concatenate(
            [bq.reshape(P, 1), bk.reshape(P, 1), c1, wsm, ii, j0pair], axis=1
        ).astype(np.float32)
    )
    cst16 = np.ascontiguousarray(pmat.astype(bf))

    # weights: wq chunks then wk chunks, [P, 4*MD]
    wq_h = Wq.reshape(DCH, P, MD).transpose(1, 0, 2).reshape(P, DCH * MD)
    wk_h = Wk.reshape(DCH, P, MD).transpose(1, 0, 2).reshape(P, DCH * MD)
    w2_h = np.ascontiguousarray(np.concatenate([wq_h, wk_h], axis=1)).astype(bf)

    in_maps = []
    for core in range(NCORES):
        xb = x[core]  # [N, D]
        # xt[n4, p, c*512 + j] = x[n4*512 + j, c*128 + p]
        xt_h = np.ascontiguousarray(
            xb.T.reshape(DCH, P, NPC, PROJ_CHUNK)
            .transpose(2, 1, 0, 3)
            .reshape(NPC, P, DCH * PROJ_CHUNK)
        ).astype(bf)
        in_maps.append({"xt": xt_h, "w2": w2_h, "cst": cst, "cst16": cst16})

    res = run_bass_kernel_spmd(nc, in_maps, list(range(NCORES)))
    last_run = (nc, in_maps)
    # y[p, t] = out[128t + p]  ->  out = y.T.flatten()
    out = np.stack(
        [res.results[c]["y"].T.reshape(-1) for c in range(NCORES)], axis=0
    )
    return out.astype(np.float32)
